# revision 15
# baseline (speedup 1.0000x reference)
"""Trainium2 Bass kernel for nn_ContrastLoss (smooth-histogram contrast loss).

Algorithm
---------
reference computes, per image:  hist[b] = sum_p w(x_p,b) / (S_p + 1e-8),
w = exp(-0.5*((x - c_b)/sigma)^2), c_b = b/255, sigma = 0.01, S_p = sum_b w,
followed by MSEs between the three histograms.

hist is a fixed linear map of the count histogram of u = round(x * 255)
in [0, 255] (256 levels = the bin centers themselves; quantization error on
the loss is ~5e-4 rel, far inside tolerance):
    hist[b] = sum_u cnt[u] * Phi[u, b]
The device only needs cnt[256] per image — a pure counting problem.

Device kernel (SPMD over 8 cores, data-parallel over pixels):
  - per core/image, 32768 pixels in SBUF [128, 256]; u = round(255 x) via the
    2^23 magic-add on ACT; split u = 16*hi + lo (hi via a second magic-add on
    ACT, lo via one DVE scalar_tensor_tensor, both exact small ints in bf16).
  - counting via PE outer products, NG=8 pixel columns block-diagonal per
    matmul m: ps += onehot_lo(group m)^T @ onehot_hi(cols of m).
    Weights APs must collapse to ONE packed free dim, so onehot_lo lives as
    Olo[p, m, l, g] (l-major inside each 8-column group): [8,16]x[1,8]
    collapses to a 128-long stride-1 run.  The moving operand tolerates a
    strided AP, so onehot_hi lives column-last as Ohi[p, w, c].  The PSUM
    table comes out index-permuted (ps[8l+g, 16g+h]) — host unscrambles.
  - BOTH one-hot layouts give batched DVE is_equal instructions whose
    operands are all 2-byte, SBUF, innermost-stride-1 -> DVE 2x_1p perf mode
    (0.52 ns/elem).  Pool builds the last 40 hi columns via per-column
    tensor_scalar (f32 comparand) to offload DVE; ACT only does prep + the
    PSUM->SBUF copy.
  - DMAs: one tiny f32 iota seed (issued on the DVE queue, which then
    derives the bf16 iota tiles on-device during its idle head), x image 0
    alone (critical path) then images 1+2 in one DMACopy — each DMACopy
    costs ~625ns on the shared HWDGE device, so fewer + smaller is faster.
  - host sums the 8 diagonal blocks of the permuted table (and the 8 cores —
    the all-reduce), applies the exact f64 cell-averaged Phi map, then MSE.
"""

import os
import sys

import numpy as np

for _p in ("/opt/trn_rl_repo", "/root/.axon_site/_ro/trn_rl_repo"):
    if os.path.isdir(_p) and _p not in sys.path:
        sys.path.insert(0, _p)

import concourse.bass as bass  # noqa: E402
import concourse.tile as tile  # noqa: E402
from concourse import bacc, mybir  # noqa: E402
from concourse.bass_utils import run_bass_kernel_spmd, axon_active  # noqa: E402

N_CORES = 8
N_IMG = 3
IMG_PIX = 4 * 1 * 256 * 256          # 262144 pixels per image
SHARD = IMG_PIX // N_CORES           # 32768 pixels per core per image
P, T = 128, 256                      # on-chip pixel layout (SHARD = P*T)
W = 16                               # one-hot width (hi and lo)
NG = 8                               # pixel columns per matmul (block-diag)
NGRP = T // NG                       # 32 column groups per image
GRID = W * W                         # 256 fine levels, u = W*hi + lo
SCALE = 255.0                        # u = round(x * 255): exactly the bins
MAGIC = 8388608.0                    # 2**23: float32 round-to-nearest trick
TC = 64                              # hi columns per DVE build instruction
G_COLS = 40                          # trailing hi columns built on Pool
MCHUNK = 16                          # lo groups per DVE build instruction
SIGMA = 0.01
BINS = 256

_CACHE = {}


def _build_program():
    nc = bacc.Bacc(
        "TRN2",
        target_bir_lowering=False,
        debug=not axon_active(),
        num_devices=N_CORES,
    )
    f32 = mybir.dt.float32
    bf16 = mybir.dt.bfloat16
    A = mybir.AluOpType
    CP = mybir.ActivationFunctionType.Copy

    x_d = nc.dram_tensor("x", [N_IMG, P, T], f32, kind="ExternalInput")
    seed_d = nc.dram_tensor("seed", [P, W], f32, kind="ExternalInput")
    cnt_d = nc.dram_tensor("cnt", [N_IMG, NG * W, NG * W], f32, kind="ExternalOutput")

    with tile.TileContext(nc) as tc:
        with (
            tc.tile_pool(name="pool", bufs=3) as pool,
            tc.tile_pool(name="cpool", bufs=1) as cpool,
            tc.tile_pool(name="psum", bufs=2, space=bass.MemorySpace.PSUM) as pp,
        ):
            # tiny seed DMA on the DVE queue; DVE then derives the iota
            # tiles during its otherwise-idle head (before image 0 prep
            # lands).  iotaRs (f32, for Pool's comparisons) IS the seed.
            iotaRs = cpool.tile([P, W], f32, tag="iotaRs")
            nc.scalar.dma_start(iotaRs[:], seed_d[:])
            iotaWb = cpool.tile([P, W], bf16, tag="iotaWb")
            nc.vector.tensor_scalar(iotaWb[:], iotaRs[:], 1.0, None, A.mult)
            iotaL2 = cpool.tile([P, W, NG], bf16, tag="iotaL2")
            nc.vector.tensor_scalar(
                iotaL2[:],
                iotaWb[:, :, None].broadcast_to([P, W, NG]),
                1.0, None, A.mult,
            )
            iotaH = cpool.tile([P, W, TC], bf16, tag="iotaH")
            nc.vector.tensor_scalar(
                iotaH[:],
                iotaWb[:, :, None].broadcast_to([P, W, TC]),
                1.0, None, A.mult,
            )

            xs, hfs, hbs, lbs = {}, {}, {}, {}
            for i in range(N_IMG):
                xs[i] = pool.tile([P, T], f32, tag="x", name=f"x{i}")
                nc.sync.dma_start(xs[i][:], x_d[i])

            def stage_a(i, halves=1):
                # u = round(x*255), hi = round((u-7.5)/16) (magic-adds, ACT);
                # lo = u - 16*hi (DVE stt).  hi kept in f32 (Pool comparand)
                # and bf16 (DVE comparand); lo in bf16.  halves=2 pipelines
                # the chain at half-tile granularity (image 0: cuts the head).
                t0 = pool.tile([P, T], f32, tag="t0", name=f"t0_{i}")
                u = pool.tile([P, T], f32, tag="u", name=f"u{i}")
                t1 = pool.tile([P, T], f32, tag="t1", name=f"t1_{i}")
                t2 = pool.tile([P, T], f32, tag="t2", name=f"t2_{i}")
                hfs[i] = pool.tile([P, T], f32, tag="hf", name=f"hf{i}")
                hbs[i] = pool.tile([P, T], bf16, tag="hb", name=f"hb{i}")
                lbs[i] = pool.tile([P, T], bf16, tag="lb", name=f"lb{i}")
                hw = T // halves
                for h in range(halves):
                    s = slice(h * hw, (h + 1) * hw)
                    nc.scalar.activation(t0[:, s], xs[i][:, s], CP, bias=MAGIC, scale=SCALE)
                    nc.scalar.activation(u[:, s], t0[:, s], CP, bias=-MAGIC)
                    nc.scalar.activation(
                        t1[:, s], u[:, s], CP, bias=8.0 - (W / 2.0 - 0.5) / W, scale=1.0 / W
                    )
                    nc.scalar.activation(t2[:, s], t1[:, s], CP, bias=MAGIC)
                    nc.scalar.activation(hfs[i][:, s], t2[:, s], CP, bias=-(MAGIC + 8.0))
                    nc.scalar.activation(hbs[i][:, s], hfs[i][:, s], CP, bias=0.0)
                    nc.vector.scalar_tensor_tensor(
                        lbs[i][:, s], hfs[i][:, s], -float(W), u[:, s], A.mult, A.add
                    )

            def stage_bc(i):
                hf, hb, lb = hfs[i], hbs[i], lbs[i]
                # lo one-hot in weights layout [P, m, l, g]: all-bf16 packed
                Olo = pool.tile([P, NGRP, W, NG], bf16, tag="Olo")
                lbg = lb.rearrange("p (m g) -> p m g", g=NG)
                for m0 in range(0, NGRP, MCHUNK):
                    nc.vector.tensor_tensor(
                        Olo[:, m0 : m0 + MCHUNK, :, :],
                        iotaL2[:, None, :, :].broadcast_to([P, MCHUNK, W, NG]),
                        lbg[:, m0 : m0 + MCHUNK, None, :].broadcast_to(
                            [P, MCHUNK, W, NG]
                        ),
                        A.is_equal,
                    )
                # hi one-hot, column-last [P, w, c]: Pool head + DVE 2x rest
                Ohi = pool.tile([P, W, T], bf16, tag="Ohi")
                for c in range(0, G_COLS):
                    nc.gpsimd.tensor_scalar(
                        Ohi[:, :, c], iotaRs[:], hf[:, c : c + 1],
                        None, A.is_equal,
                    )
                for c0 in range(G_COLS, T, TC):
                    n = min(TC, T - c0)
                    nc.vector.tensor_tensor(
                        Ohi[:, :, c0 : c0 + n],
                        iotaH[:, :, 0:n],
                        hb[:, None, c0 : c0 + n].broadcast_to([P, W, n]),
                        A.is_equal,
                    )

                ps = pp.tile([NG * W, NG * W], f32, tag="ps")
                for m in range(NGRP):
                    lhsT = Olo[:, m, :, :]
                    rhs = Ohi[:, :, m * NG : (m + 1) * NG].rearrange(
                        "p w c -> p c w"
                    )
                    nc.tensor.matmul(
                        ps[:], lhsT, rhs, start=(m == 0), stop=(m == NGRP - 1)
                    )
                res = pool.tile([NG * W, NG * W], f32, tag="res")
                nc.scalar.activation(res[:], ps[:], CP, bias=0.0)
                nc.sync.dma_start(cnt_d[i], res[:])

            # logical timestamps order each image's chain for the tile
            # scheduler (image i strictly before image i+1 per engine)
            with tc.tile_wait_until(0):
                stage_a(0, halves=2)
            with tc.tile_wait_until(0.5):
                stage_bc(0)
            with tc.tile_wait_until(1):
                stage_a(1)
            with tc.tile_wait_until(1.5):
                stage_bc(1)
            with tc.tile_wait_until(2):
                stage_a(2)
            with tc.tile_wait_until(2.5):
                stage_bc(2)

    nc.compile()
    return nc


def _phi():
    """f64 [GRID, BINS] map: cell-averaged smooth-histogram contribution."""
    b = np.arange(BINS, dtype=np.float64)
    step = SCALE / 255.0
    u_grid = np.arange(GRID, dtype=np.float64)
    nsub = 17
    offs = np.linspace(-0.5, 0.5, nsub)
    wts = np.ones(nsub)
    wts[1:-1:2], wts[2:-1:2] = 4.0, 2.0
    wts /= wts.sum()
    phi = np.zeros((GRID, BINS))
    for o, ws in zip(offs, wts):
        diff = ((u_grid + o)[:, None] - step * b[None, :]) / SCALE
        w = np.exp(-0.5 * (diff / SIGMA) ** 2)
        phi += ws * (w / (w.sum(axis=1, keepdims=True) + 1e-8))
    return phi


def _seed_np():
    return np.ascontiguousarray(
        np.broadcast_to(np.arange(W, dtype=np.float32)[None, :], (P, W))
    )


def _get_state():
    if "nc" not in _CACHE:
        _CACHE["nc"] = _build_program()
        _CACHE["phi"] = _phi()
        _CACHE["seed"] = _seed_np()
    return _CACHE["nc"], _CACHE["phi"], _CACHE["seed"]


def _run_device(images, trace=False):
    """images: [3, IMG_PIX] f32 -> (results, counts [3, GRID] f64)."""
    nc, phi, seed = _get_state()
    in_maps = []
    for k in range(N_CORES):
        shard = images[:, k * SHARD : (k + 1) * SHARD].reshape(N_IMG, P, T)
        in_maps.append({"x": np.ascontiguousarray(shard), "seed": seed})
    res = run_bass_kernel_spmd(nc, in_maps, list(range(N_CORES)), trace=trace)
    cnt = np.zeros((N_IMG, GRID), dtype=np.float64)
    for k in range(N_CORES):
        ps = res.results[k]["cnt"].astype(np.float64)  # [3, 128, 128]
        # ps[8l+g, 16g+h] -> cnt[u = 16h+l]
        psr = ps.reshape(N_IMG, W, NG, NG, W)  # [i, l, g, g', h]
        for g in range(NG):
            cnt += psr[:, :, g, g, :].transpose(0, 2, 1).reshape(N_IMG, GRID)
    return res, cnt


def kernel(fused_image, ir_image, visible_gray):
    imgs = np.stack(
        [
            np.asarray(fused_image, dtype=np.float32).reshape(-1),
            np.asarray(ir_image, dtype=np.float32).reshape(-1),
            np.asarray(visible_gray, dtype=np.float32).reshape(-1),
        ]
    )
    _, cnt = _run_device(imgs)
    _, phi, _ = _get_state()
    hists = cnt @ phi  # [3, 256] f64
    hf, hi_, hv = hists
    loss_ir = np.mean((hf - hi_) ** 2)
    loss_vis = np.mean((hf - hv) ** 2)
    return np.array(0.5 * loss_ir + 0.5 * loss_vis, dtype=np.float32)


# revision 16
# speedup vs baseline: 1.4234x; 1.4234x over previous
"""Trainium2 Bass kernel for nn_ContrastLoss (smooth-histogram contrast loss).

Algorithm
---------
reference computes, per image:  hist[b] = sum_p w(x_p,b) / (S_p + 1e-8),
w = exp(-0.5*((x - c_b)/sigma)^2), c_b = b/255, sigma = 0.01, S_p = sum_b w,
followed by MSEs between the three histograms.

hist is a fixed linear map of the count histogram of u = round(x * 255)
in [0, 255] (256 levels = the bin centers themselves; quantization error on
the loss is ~5e-4 rel, far inside tolerance):
    hist[b] = sum_u cnt[u] * Phi[u, b]
The device only needs cnt[256] per image — a pure counting problem.

Device kernel (SPMD over 8 cores, data-parallel over pixels):
  - per core/image, 32768 pixels in SBUF [128, 256]; u = round(255 x) via the
    2^23 magic-add on ACT; split u = 16*hi + lo (hi via a second magic-add on
    ACT, lo via one DVE scalar_tensor_tensor, both exact small ints in bf16).
  - counting via PE outer products, NG=8 pixel columns block-diagonal per
    matmul m: ps += onehot_lo(group m)^T @ onehot_hi(cols of m).
    Weights APs must collapse to ONE packed free dim, so onehot_lo lives as
    Olo[p, m, l, g] (l-major inside each 8-column group): [8,16]x[1,8]
    collapses to a 128-long stride-1 run.  The moving operand tolerates a
    strided AP, so onehot_hi lives column-last as Ohi[p, w, c].  The PSUM
    table comes out index-permuted (ps[8l+g, 16g+h]) — host unscrambles.
  - BOTH one-hot layouts give batched DVE is_equal instructions whose
    operands are all 2-byte, SBUF, innermost-stride-1 -> DVE 2x_1p perf mode
    (0.52 ns/elem).  Pool builds the last 40 hi columns via per-column
    tensor_scalar (f32 comparand) to offload DVE; ACT only does prep + the
    PSUM->SBUF copy.
  - DMAs: one tiny f32 iota seed (issued on the DVE queue, which then
    derives the bf16 iota tiles on-device during its idle head), x image 0
    alone (critical path) then images 1+2 in one DMACopy — each DMACopy
    costs ~625ns on the shared HWDGE device, so fewer + smaller is faster.
  - host sums the 8 diagonal blocks of the permuted table (and the 8 cores —
    the all-reduce), applies the exact f64 cell-averaged Phi map, then MSE.
"""

import os
import sys

import numpy as np

for _p in ("/opt/trn_rl_repo", "/root/.axon_site/_ro/trn_rl_repo"):
    if os.path.isdir(_p) and _p not in sys.path:
        sys.path.insert(0, _p)

import concourse.bass as bass  # noqa: E402
import concourse.tile as tile  # noqa: E402
from concourse import bacc, mybir  # noqa: E402
from concourse.bass_utils import run_bass_kernel_spmd, axon_active  # noqa: E402

N_CORES = 8
N_IMG = 3
IMG_PIX = 4 * 1 * 256 * 256          # 262144 pixels per image
SHARD = IMG_PIX // N_CORES           # 32768 pixels per core per image
P, T = 128, 256                      # on-chip pixel layout (SHARD = P*T)
W = 16                               # one-hot width (hi and lo)
NG = 8                               # pixel columns per matmul (block-diag)
NGRP = T // NG                       # 32 column groups per image
GRID = W * W                         # 256 fine levels, u = W*hi + lo
SCALE = 255.0                        # u = round(x * 255): exactly the bins
MAGIC = 8388608.0                    # 2**23: float32 round-to-nearest trick
TC = 64                              # hi columns per DVE build instruction
G_COLS = 40                          # trailing hi columns built on Pool
MCHUNK = 16                          # lo groups per DVE build instruction
SIGMA = 0.01
BINS = 256

_CACHE = {}


def _build_program():
    nc = bacc.Bacc(
        "TRN2",
        target_bir_lowering=False,
        debug=not axon_active(),
        num_devices=N_CORES,
    )
    f32 = mybir.dt.float32
    bf16 = mybir.dt.bfloat16
    A = mybir.AluOpType
    CP = mybir.ActivationFunctionType.Copy

    x_d = nc.dram_tensor("x", [N_IMG, P, T], f32, kind="ExternalInput")
    seed_d = nc.dram_tensor("seed", [P, W], f32, kind="ExternalInput")
    cnt_d = nc.dram_tensor("cnt", [N_IMG, NG * W, NG * W], f32, kind="ExternalOutput")

    with tile.TileContext(nc) as tc:
        with (
            tc.tile_pool(name="pool", bufs=3) as pool,
            tc.tile_pool(name="cpool", bufs=1) as cpool,
            tc.tile_pool(name="psum", bufs=2, space=bass.MemorySpace.PSUM) as pp,
        ):
            # tiny seed DMA on the DVE queue; DVE then derives the iota
            # tiles during its otherwise-idle head (before image 0 prep
            # lands).  iotaRs (f32, for Pool's comparisons) IS the seed.
            iotaRs = cpool.tile([P, W], f32, tag="iotaRs")
            nc.scalar.dma_start(iotaRs[:], seed_d[:])
            iotaWb = cpool.tile([P, W], bf16, tag="iotaWb")
            nc.vector.tensor_scalar(iotaWb[:], iotaRs[:], 1.0, None, A.mult)
            iotaL2 = cpool.tile([P, W, NG], bf16, tag="iotaL2")
            nc.vector.tensor_scalar(
                iotaL2[:],
                iotaWb[:, :, None].broadcast_to([P, W, NG]),
                1.0, None, A.mult,
            )
            iotaH = cpool.tile([P, W, TC], bf16, tag="iotaH")
            nc.vector.tensor_scalar(
                iotaH[:],
                iotaWb[:, :, None].broadcast_to([P, W, TC]),
                1.0, None, A.mult,
            )

            xs, hfs, hbs, lbs = {}, {}, {}, {}
            for i in range(N_IMG):
                xs[i] = pool.tile([P, T], f32, tag="x", name=f"x{i}")
                nc.sync.dma_start(xs[i][:], x_d[i])

            def stage_a(i, halves=1):
                # u = round(x*255), hi = round((u-7.5)/16) (magic-adds, ACT);
                # lo = u - 16*hi (DVE stt).  hi kept in f32 (Pool comparand)
                # and bf16 (DVE comparand); lo in bf16.  halves=2 pipelines
                # the chain at half-tile granularity (image 0: cuts the head).
                t0 = pool.tile([P, T], f32, tag="t0", name=f"t0_{i}")
                u = pool.tile([P, T], f32, tag="u", name=f"u{i}")
                t1 = pool.tile([P, T], f32, tag="t1", name=f"t1_{i}")
                t2 = pool.tile([P, T], f32, tag="t2", name=f"t2_{i}")
                hfs[i] = pool.tile([P, T], f32, tag="hf", name=f"hf{i}")
                hbs[i] = pool.tile([P, T], bf16, tag="hb", name=f"hb{i}")
                lbs[i] = pool.tile([P, T], bf16, tag="lb", name=f"lb{i}")
                hw = T // halves
                for h in range(halves):
                    s = slice(h * hw, (h + 1) * hw)
                    nc.scalar.activation(t0[:, s], xs[i][:, s], CP, bias=MAGIC, scale=SCALE)
                    nc.scalar.activation(u[:, s], t0[:, s], CP, bias=-MAGIC)
                    nc.scalar.activation(
                        t1[:, s], u[:, s], CP, bias=8.0 - (W / 2.0 - 0.5) / W, scale=1.0 / W
                    )
                    nc.scalar.activation(t2[:, s], t1[:, s], CP, bias=MAGIC)
                    nc.scalar.activation(hfs[i][:, s], t2[:, s], CP, bias=-(MAGIC + 8.0))
                    nc.scalar.activation(hbs[i][:, s], hfs[i][:, s], CP, bias=0.0)
                    nc.vector.scalar_tensor_tensor(
                        lbs[i][:, s], hfs[i][:, s], -float(W), u[:, s], A.mult, A.add
                    )

            def stage_bc(i):
                hf, hb, lb = hfs[i], hbs[i], lbs[i]
                # lo one-hot in weights layout [P, m, l, g]: all-bf16 packed
                Olo = pool.tile([P, NGRP, W, NG], bf16, tag="Olo")
                lbg = lb.rearrange("p (m g) -> p m g", g=NG)
                for m0 in range(0, NGRP, MCHUNK):
                    nc.vector.tensor_tensor(
                        Olo[:, m0 : m0 + MCHUNK, :, :],
                        iotaL2[:, None, :, :].broadcast_to([P, MCHUNK, W, NG]),
                        lbg[:, m0 : m0 + MCHUNK, None, :].broadcast_to(
                            [P, MCHUNK, W, NG]
                        ),
                        A.is_equal,
                    )
                # hi one-hot, column-last [P, w, c]: Pool head + DVE 2x rest
                Ohi = pool.tile([P, W, T], bf16, tag="Ohi")
                for c in range(0, G_COLS):
                    nc.gpsimd.tensor_scalar(
                        Ohi[:, :, c], iotaRs[:], hf[:, c : c + 1],
                        None, A.is_equal,
                    )
                for c0 in range(G_COLS, T, TC):
                    n = min(TC, T - c0)
                    nc.vector.tensor_tensor(
                        Ohi[:, :, c0 : c0 + n],
                        iotaH[:, :, 0:n],
                        hb[:, None, c0 : c0 + n].broadcast_to([P, W, n]),
                        A.is_equal,
                    )

                ps = pp.tile([NG * W, NG * W], f32, tag="ps")
                for m in range(NGRP):
                    lhsT = Olo[:, m, :, :]
                    rhs = Ohi[:, :, m * NG : (m + 1) * NG].rearrange(
                        "p w c -> p c w"
                    )
                    nc.tensor.matmul(
                        ps[:], lhsT, rhs, start=(m == 0), stop=(m == NGRP - 1)
                    )
                res = pool.tile([NG * W, NG * W], f32, tag="res")
                nc.scalar.activation(res[:], ps[:], CP, bias=0.0)
                nc.sync.dma_start(cnt_d[i], res[:])

            # logical timestamps order each image's chain for the tile
            # scheduler (image i strictly before image i+1 per engine)
            US = 0.001  # tile_wait_until is in ms; nudge at us scale
            with tc.tile_wait_until(0 * US):
                stage_a(0, halves=2)
            with tc.tile_wait_until(0.5 * US):
                stage_bc(0)
            with tc.tile_wait_until(4 * US):
                stage_a(1)
            with tc.tile_wait_until(5 * US):
                stage_bc(1)
            with tc.tile_wait_until(9 * US):
                stage_a(2)
            with tc.tile_wait_until(10 * US):
                stage_bc(2)

    nc.compile()
    return nc


def _phi():
    """f64 [GRID, BINS] map: cell-averaged smooth-histogram contribution."""
    b = np.arange(BINS, dtype=np.float64)
    step = SCALE / 255.0
    u_grid = np.arange(GRID, dtype=np.float64)
    nsub = 17
    offs = np.linspace(-0.5, 0.5, nsub)
    wts = np.ones(nsub)
    wts[1:-1:2], wts[2:-1:2] = 4.0, 2.0
    wts /= wts.sum()
    phi = np.zeros((GRID, BINS))
    for o, ws in zip(offs, wts):
        diff = ((u_grid + o)[:, None] - step * b[None, :]) / SCALE
        w = np.exp(-0.5 * (diff / SIGMA) ** 2)
        phi += ws * (w / (w.sum(axis=1, keepdims=True) + 1e-8))
    return phi


def _seed_np():
    return np.ascontiguousarray(
        np.broadcast_to(np.arange(W, dtype=np.float32)[None, :], (P, W))
    )


def _get_state():
    if "nc" not in _CACHE:
        _CACHE["nc"] = _build_program()
        _CACHE["phi"] = _phi()
        _CACHE["seed"] = _seed_np()
    return _CACHE["nc"], _CACHE["phi"], _CACHE["seed"]


def _run_device(images, trace=False):
    """images: [3, IMG_PIX] f32 -> (results, counts [3, GRID] f64)."""
    nc, phi, seed = _get_state()
    in_maps = []
    for k in range(N_CORES):
        shard = images[:, k * SHARD : (k + 1) * SHARD].reshape(N_IMG, P, T)
        in_maps.append({"x": np.ascontiguousarray(shard), "seed": seed})
    res = run_bass_kernel_spmd(nc, in_maps, list(range(N_CORES)), trace=trace)
    cnt = np.zeros((N_IMG, GRID), dtype=np.float64)
    for k in range(N_CORES):
        ps = res.results[k]["cnt"].astype(np.float64)  # [3, 128, 128]
        # ps[8l+g, 16g+h] -> cnt[u = 16h+l]
        psr = ps.reshape(N_IMG, W, NG, NG, W)  # [i, l, g, g', h]
        for g in range(NG):
            cnt += psr[:, :, g, g, :].transpose(0, 2, 1).reshape(N_IMG, GRID)
    return res, cnt


def kernel(fused_image, ir_image, visible_gray):
    imgs = np.stack(
        [
            np.asarray(fused_image, dtype=np.float32).reshape(-1),
            np.asarray(ir_image, dtype=np.float32).reshape(-1),
            np.asarray(visible_gray, dtype=np.float32).reshape(-1),
        ]
    )
    _, cnt = _run_device(imgs)
    _, phi, _ = _get_state()
    hists = cnt @ phi  # [3, 256] f64
    hf, hi_, hv = hists
    loss_ir = np.mean((hf - hi_) ** 2)
    loss_vis = np.mean((hf - hv) ** 2)
    return np.array(0.5 * loss_ir + 0.5 * loss_vis, dtype=np.float32)
